# revision 15
# baseline (speedup 1.0000x reference)
"""DPS perturbed-top-k patch-extraction kernel for Trainium2 (Bass/Tile), v3.

Contract: kernel(**inputs) takes the FULL inputs
    x_high  (8, 3, 512, 512) f32
    scores_2d (8, 16, 16) f32
    noise   (8, 500, 256) f32
and returns the FULL output (128, 3, 64, 64) f32.

Sharding: pure data-parallel over batch b across the 8 NeuronCores.

v3 design (vs v2's 92us DRAM-staged scatter):
  * NO DRAM staging.  x loads naturally (4 big-run DMAs), engines do the
    (c,w) -> (b,c,w') column-block interleave as an SBUF free-axis
    shuffle (casting f32->bf16), then 20 small SBUF->SBUF DMAs scatter
    rows into the three blocked B tiles (partition p = 6b + a2,
    a = 6m + a2 block-row, b block-col, free = (h',c,w')).
  * B is bf16 (tolerance 2e-2; bf16 keeps rel err ~1e-3) which halves
    the PE moving-operand time of the main matmul.
  * cnt is computed compactly (256 cols, no 18-stride embedding):
    is_ge + prefix scan on [125, 256]; the embedding to the 324-wide
    d' axis happens later on the tiny [16, 256] indicator row.
  * G_k(d) = #{n: cnt[n,d] > k}: DVE does is_ge-counts on n-half 0,
    ACT does Sign-sums on n-half 1; Gc = Gv + 0.5*Sa (per-k constants
    cancel in the d-difference that forms the indicator).
  * indicators transposed back to block partitions with the p = 6b+a2
    permutation folded into 12 tiny permuted engine copies.
  * output written as (64, 3072) quadrant-major rows (one 64x12KB DMA)
    and reassembled to (16,3,64,64) on the host during unshard.
"""
import numpy as np
from contextlib import ExitStack

# ---- problem constants (hardcoded per spec) ----
NB = 8
C = 3
H = W = 512
HW = H * W
GS = 16
GE = 18          # embedded grid stride (d' = 18i + j)
D2 = 256
D3 = GE * GE     # 324
K = 16
N = 500
NCH = 4
NP = 125
CM = 108         # B partitions per tile (18b x 6a2)
PATCH = 64
BLK = 32
SIG = 0.05
INV_N = 1.0 / 500.0
NEG = -1.0e30
F = C * BLK * BLK      # 3072 floats per block partition
XSROW = GE * 96        # 1728: xs free width (18 b-slots x 96)

_CACHE = {}


def _build_nc():
    import concourse.bacc as bacc
    import concourse.bass as bass
    import concourse.mybir as mybir
    import concourse.tile as tile

    F32 = mybir.dt.float32
    BF16 = mybir.dt.bfloat16
    I32 = mybir.dt.int32
    ALU = mybir.AluOpType
    ACTF = mybir.ActivationFunctionType
    AP = bass.AP

    nc = bacc.Bacc("TRN2", target_bir_lowering=False, debug=False)
    x_d = nc.dram_tensor("x", (C, H, W), F32, kind="ExternalInput")
    sc_d = nc.dram_tensor("sc", (GS, GS), F32, kind="ExternalInput")
    nz_d = nc.dram_tensor("nz", (N, D2), F32, kind="ExternalInput")
    o_d = nc.dram_tensor("o", (64, F), F32, kind="ExternalOutput")

    with tile.TileContext(nc) as tc, ExitStack() as ctx:
        sb = ctx.enter_context(tc.tile_pool(name="sb", bufs=1))
        ps_rep = ctx.enter_context(tc.tile_pool(name="ps_rep", bufs=1, space="PSUM"))
        ps_cnt = ctx.enter_context(tc.tile_pool(name="ps_cnt", bufs=1, space="PSUM"))
        ps_out = ctx.enter_context(tc.tile_pool(name="ps_out", bufs=3, space="PSUM"))

        def ap_of(t, off_elems, dims):
            return AP(t.tensor, t[:].offset + off_elems, dims)

        dma_s = nc.sync.dma_start
        dma_a = nc.scalar.dma_start
        dma_g = nc.gpsimd.dma_start

        # ---------------- natural x loads (first thing) -----------------
        xn = [sb.tile([128, 1536], F32, tag=f"xn{t}", name=f"xn{t}")
              for t in range(4)]
        for t in range(4):
            (dma_s if t % 2 == 0 else dma_a)(
                xn[t][:],
                AP(x_d, t * 128 * W, [[W, 128], [HW, 3], [1, 512]]))

        # noise + scores early on act queue
        nz_t = [sb.tile([128, D2], F32, tag=f"nz{t}", name=f"nzt{t}")
                for t in range(NCH)]
        dma_a(nz_t[0][0:NP, :], nz_d[0:NP, :])
        s256 = sb.tile([1, D2], F32)
        dma_a(s256[:], sc_d[:].rearrange("a b -> (a b)").unsqueeze(0))
        for t in range(1, NCH):
            dma_a(nz_t[t][0:NP, :], nz_d[NP * t:NP * (t + 1), :])

        # ---------------- constants ----------------
        iota_t = sb.tile([128, 128], I32)
        nc.gpsimd.iota(iota_t[:], pattern=[[-1, 128]], base=0,
                       channel_multiplier=1)
        ident = sb.tile([128, 128], BF16)
        nc.vector.tensor_scalar(ident[:], iota_t[:], 0, None, op0=ALU.is_equal)
        diag05 = sb.tile([128, 128], F32)
        nc.vector.tensor_scalar(diag05[:], iota_t[:], 0, SIG,
                                op0=ALU.is_equal, op1=ALU.mult)
        ident_f32 = sb.tile([128, 128], F32)
        nc.vector.tensor_scalar(ident_f32[:], iota_t[:], 0, None,
                                op0=ALU.is_equal)
        # bias table for ACT Sign form: col j = -(j+0.5)  (iota_r[p,j] = -j)
        iota_r = sb.tile([128, 17], I32)
        nc.gpsimd.iota(iota_r[:], pattern=[[-1, 17]], base=0,
                       channel_multiplier=0)
        bias_f = sb.tile([128, 17], F32)
        nc.vector.tensor_scalar(bias_f[:], iota_r[:], 1.0, -0.5,
                                op0=ALU.mult, op1=ALU.add)
        ones = sb.tile([1, 128], F32)
        nc.vector.memset(ones[:], 1.0)

        # ---------------- B tiles (bf16) + memsets ----------------------
        B = [sb.tile([CM, F], BF16, tag=f"B{m}", name=f"B{m}") for m in range(3)]
        nc.vector.memset(B[0][:], 0.0)
        nc.vector.memset(B[1][:], 0.0)
        nc.gpsimd.memset(B[2][:], 0.0)

        # ---------------- scores normalization (DVE) --------------------
        smax = sb.tile([1, 1], F32)
        smin = sb.tile([1, 1], F32)
        nc.vector.tensor_reduce(smax[:], s256[:], axis=mybir.AxisListType.X,
                                op=ALU.max)
        nc.vector.tensor_reduce(smin[:], s256[:], axis=mybir.AxisListType.X,
                                op=ALU.min)
        Dt = sb.tile([1, 1], F32)
        nc.vector.tensor_scalar(Dt[:], smax[:], smin[:], 1e-5,
                                op0=ALU.subtract, op1=ALU.add)
        rD = sb.tile([1, 1], F32)
        nc.vector.reciprocal(rD[:], Dt[:])
        s_row = sb.tile([1, D2], F32)
        nc.vector.tensor_scalar(s_row[:], s256[:], smin[:], rD[:],
                                op0=ALU.subtract, op1=ALU.mult)

        # ---------------- xs shuffle: (c,w) -> (b,c,w') bf16 ------------
        xs = [sb.tile([128, XSROW], BF16, tag=f"xs{t}", name=f"xs{t}")
              for t in range(4)]
        cp = {0: nc.vector.tensor_copy, 2: nc.vector.tensor_copy,
              1: lambda d, s_: nc.scalar.copy(d, s_),
              3: nc.gpsimd.tensor_copy}
        ms = {0: nc.vector.memset, 2: nc.vector.memset,
              1: nc.gpsimd.memset, 3: nc.gpsimd.memset}
        for t in range(4):
            # pad strips: b0 w'<16 per c; b16 w'>=16 per c; b17 fully
            ms[t](ap_of(xs[t], 0, [[XSROW, 128], [32, 3], [1, 16]]), 0.0)
            ms[t](ap_of(xs[t], 16 * 96 + 16, [[XSROW, 128], [32, 3], [1, 16]]),
                  0.0)
            ms[t](xs[t][:, 17 * 96:XSROW], 0.0)
            for c in range(C):
                # interior: cols 16..495 -> b 1..15
                cp[t](ap_of(xs[t], 96 + 32 * c, [[XSROW, 128], [96, 15], [1, 32]]),
                      ap_of(xn[t], 512 * c + 16, [[1536, 128], [32, 15], [1, 32]]))
            # left edge cols 0..15 -> b0 w' 16..31 (all c)
            cp[t](ap_of(xs[t], 16, [[XSROW, 128], [32, 3], [1, 16]]),
                  ap_of(xn[t], 0, [[1536, 128], [512, 3], [1, 16]]))
            # right edge cols 496..511 -> b16 w' 0..15
            cp[t](ap_of(xs[t], 16 * 96, [[XSROW, 128], [32, 3], [1, 16]]),
                  ap_of(xn[t], 496, [[1536, 128], [512, 3], [1, 16]]))

        # ---------------- B fill via block-major DRAM xq ----------------
        # xq[m] bf16 layout [b][a2][h'][c,w']: row r maps affinely to
        # offset b*18432 + ((r+16)-192m)*96, so each (m, xs-tile) overlap
        # segment is ONE 3-dim scatter DMA (src partition-first).  B then
        # loads with 3 contiguous partition-first DMAs (p = 6b + a2).
        SLAB2 = 6 * F            # 18432 elems per b-slab
        xq = [nc.dram_tensor(f"xq{m}", (18 * SLAB2,), BF16, kind="Internal")
              for m in range(3)]
        # row-pad zero-fills (col pads already zeroed in xs strips)
        zb = sb.tile([128, 512], BF16)
        nc.vector.memset(zb[:], 0.0)
        # m0: a2=0 h'<16; m2: a2=4 h'>=16; m2: a2=5 fully
        dma_s(AP(xq[0], 0, [[SLAB2, 18], [512, 3], [1, 512]]),
              ap_of(zb, 0, [[512, 18], [0, 3], [1, 512]]))
        dma_s(AP(xq[2], 4 * F + 1536, [[SLAB2, 18], [512, 3], [1, 512]]),
              ap_of(zb, 0, [[512, 18], [0, 3], [1, 512]]))
        dma_s(AP(xq[2], 5 * F, [[SLAB2, 18], [512, 6], [1, 512]]),
              ap_of(zb, 0, [[512, 18], [0, 6], [1, 512]]))
        # scatter xs -> xq: segments of (m-range x xs-tile-range)
        segs = []
        for m in range(3):
            mlo, mhi = max(192 * m - 16, 0), min(192 * (m + 1) - 16, 512)
            seg = mlo
            while seg < mhi:
                t = seg // 128
                send = min(mhi, 128 * (t + 1))
                segs.append((m, t, seg, send))
                seg = send
        for m, t, r0, r1 in segs:
            nr = r1 - r0
            dst = AP(xq[m], ((r0 + 16) - 192 * m) * 96,
                     [[96, nr], [SLAB2, 18], [1, 96]])
            src = ap_of(xs[t], (r0 - 128 * t) * XSROW,
                        [[XSROW, nr], [96, 18], [1, 96]])
            dma_g(dst, src)
        # hop2: contiguous B loads
        for m in range(3):
            (dma_s if m != 1 else dma_a)(
                ap_of(B[m], 0, [[F, CM], [1, F]]),
                AP(xq[m], 0, [[F, CM], [1, F]]))

        # ---------------- noise top-k -> compact cnt --------------------
        cnt = [sb.tile([128, D2], BF16, tag=f"cnt{t}", name=f"cnt{t}")
               for t in range(NCH)]
        for t in range(NCH):
            pert_ps = ps_rep.tile([128, D2], F32, tag="pert_ps",
                                  name=f"pert_ps{t}", bufs=2)
            nc.tensor.matmul(pert_ps[0:NP, :], ones[:, 0:NP], s_row[:],
                             start=True, stop=False)
            nc.tensor.matmul(pert_ps[0:NP, :], diag05[0:NP, 0:NP],
                             nz_t[t][0:NP, :], start=False, stop=True)
            pert = sb.tile([128, D2], F32, tag=f"pert{t}", name=f"pert{t}")
            if t % 2 == 0:
                nc.scalar.copy(pert[0:NP, :], pert_ps[0:NP, :])
            else:
                nc.vector.tensor_copy(pert[0:NP, :], pert_ps[0:NP, :])
            top8 = sb.tile([128, 8], F32, tag=f"top8{t}", name=f"top8_{t}")
            nc.vector.max(top8[0:NP, :], pert[0:NP, :])
            pert2 = sb.tile([128, D2], F32, tag=f"pert2{t}", name=f"pert2_{t}")
            nc.vector.match_replace(pert2[0:NP, :], top8[0:NP, :],
                                    pert[0:NP, :], NEG)
            top8b = sb.tile([128, 8], F32, tag=f"top8b{t}", name=f"top8b_{t}")
            nc.vector.max(top8b[0:NP, :], pert2[0:NP, :])
            At = sb.tile([128, D2], F32, tag=f"A{t}", name=f"A{t}")
            nc.vector.tensor_scalar(At[0:NP, :], pert[0:NP, :],
                                    top8b[0:NP, 7:8], None, op0=ALU.is_ge)
            nc.vector.memset(cnt[t][:], 0.0)
            nc.vector.tensor_tensor_scan(cnt[t][0:NP, :], At[0:NP, :],
                                         At[0:NP, :], initial=0.0,
                                         op0=ALU.add, op1=ALU.bypass)

        # compact transposes: cntT2[u] [128 d-part, 512 n] (PSUM, bf16)
        cntT2 = [ps_cnt.tile([128, 512], BF16, tag=f"cntT{u}", name=f"cntT{u}")
                 for u in range(2)]
        for t in range(NCH):
            for u in range(2):
                nc.tensor.transpose(
                    cntT2[u][:, 128 * t:128 * (t + 1)],
                    cnt[t][:, 128 * u:128 * (u + 1)], ident[:])

        # ---------------- G: split-n threshold counts -------------------
        # DVE: Gv_k = #{n in half0: cnt >= k+1}; ACT: Sa_k = sign-sum over
        # half1 (= 2*G1_k - 256).  Gc = Gv + 0.5*Sa (+128, cancels in diff)
        Gv = [sb.tile([128, K], F32, tag=f"Gv{u}", name=f"Gv{u}") for u in range(2)]
        Sa = [sb.tile([128, K], F32, tag=f"Sa{u}", name=f"Sa{u}") for u in range(2)]
        scr_v = sb.tile([128, D2], BF16, tag="scr_v", name="scr_v")
        scr_a = sb.tile([128, D2], BF16, tag="scr_a", name="scr_a")
        for u in range(2):
            for k in range(K):
                nc.vector.tensor_scalar(scr_v[:], cntT2[u][:, 0:256],
                                        float(k) + 0.5, None, op0=ALU.is_ge,
                                        op1=ALU.add,
                                        accum_out=Gv[u][:, k:k + 1])
                nc.scalar.activation(scr_a[:], cntT2[u][:, 256:512], ACTF.Sign,
                                     bias=bias_f[:, k:k + 1], scale=1.0,
                                     accum_out=Sa[u][:, k:k + 1])
        Gc = [sb.tile([128, K], F32, tag=f"Gc{u}", name=f"Gc{u}") for u in range(2)]
        for u in range(2):
            nc.vector.scalar_tensor_tensor(Gc[u][:], Sa[u][:], 0.5, Gv[u][:],
                                           op0=ALU.mult, op1=ALU.add)

        # ---------------- gct -> compact indicator ----------------------
        gct_sb = sb.tile([16, 1 + D2], F32)
        # col0 = scaled form at d=-1: (0 + 0.5*(-256)) * INV_N
        nc.vector.memset(gct_sb[:, 0:1], -128.0 * INV_N)
        for u in range(2):
            gct_ps = ps_rep.tile([16, 128], F32, tag="pert_ps",
                                 name=f"gct{u}", bufs=2)
            nc.tensor.transpose(gct_ps[:], Gc[u][:], ident_f32[:])
            nc.vector.tensor_scalar(gct_sb[:, 1 + 128 * u:1 + 128 * (u + 1)],
                                    gct_ps[:], INV_N, None, op0=ALU.mult)
        indC = sb.tile([16, D2], F32)
        nc.vector.tensor_tensor(indC[:], gct_sb[:, 1:1 + D2],
                                gct_sb[:, 0:D2], op=ALU.subtract)
        # embed into d' = 18i + j (rims stay zero)
        indT_pad = sb.tile([16, 19 + D3], F32)
        nc.vector.memset(indT_pad[:], 0.0)
        nc.vector.tensor_copy(
            ap_of(indT_pad, 19, [[19 + D3, 16], [GE, GS], [1, GS]]),
            ap_of(indC, 0, [[D2, 16], [GS, GS], [1, GS]]))

        # permuted back-transposes: element #p = ind(d'(p) - s), p = 6b+a2
        INDr = [sb.tile([CM, 64], BF16, tag=f"INDr{m}", name=f"INDr{m}")
                for m in range(3)]
        engs = [nc.vector.tensor_copy, nc.gpsimd.tensor_copy,
                lambda d, s_: nc.scalar.copy(d, s_)]
        for m in range(3):
            ind_ps = ps_cnt.tile([CM, 64], F32, tag=f"cntT{m % 2}",
                                 name=f"indps{m}")
            for hq in range(2):
                for wq in range(2):
                    q = 2 * hq + wq
                    s = GE * hq + wq
                    tmp = sb.tile([16, CM], F32, tag=f"iperm{q % 2}",
                                  name=f"iperm{m}_{q}")
                    src = AP(indT_pad.tensor,
                             indT_pad[:].offset + 19 + CM * m - s,
                             [[19 + D3, 16], [1, 18], [18, 6]])
                    engs[(2 * m + q) % 3](tmp[:], src)
                    nc.tensor.transpose(ind_ps[:, 16 * q:16 * (q + 1)],
                                        tmp[:], ident_f32[0:16, 0:16])
            if m % 2 == 0:
                nc.vector.tensor_copy(INDr[m][:], ind_ps[:])
            else:
                nc.scalar.copy(INDr[m][:], ind_ps[:])

        # ---------------- main matmul (bf16) + output -------------------
        osb = sb.tile([64, F], F32)
        for t in range(7):
            ncol = 480 if t < 6 else 192
            mm = ps_out.tile([64, 480], F32, tag="mm", name=f"mm{t}")
            for m in range(3):
                nc.tensor.matmul(mm[:, 0:ncol], INDr[m][:],
                                 B[m][:, 480 * t:480 * t + ncol],
                                 start=(m == 0), stop=(m == 2))
            dst = osb[:, 480 * t:480 * t + ncol]
            if t % 2 == 0:
                nc.scalar.copy(dst, mm[:, 0:ncol])
            else:
                nc.vector.tensor_copy(dst, mm[:, 0:ncol])
            if t == 3:
                dma_s(AP(o_d, 0, [[F, 64], [1, 1920]]),
                      ap_of(osb, 0, [[F, 64], [1, 1920]]))
        dma_s(AP(o_d, 1920, [[F, 64], [1, F - 1920]]),
              ap_of(osb, 1920, [[F, 64], [1, F - 1920]]))

    nc.compile()
    return nc


def _get_nc():
    if "nc" not in _CACHE:
        _CACHE["nc"] = _build_nc()
    return _CACHE["nc"]


def _unscramble(o2):
    # o2 (64, 3072) rows = (hq, wq, k), cols = (h', c, w')
    return (o2.reshape(2, 2, K, 32, C, 32)
              .transpose(2, 4, 0, 3, 1, 5)
              .reshape(K, C, PATCH, PATCH))


def _run(x_high, scores_2d, noise, trace=False):
    from concourse import bass_utils
    nc = _get_nc()
    x_high = np.ascontiguousarray(x_high, dtype=np.float32)
    scores_2d = np.ascontiguousarray(scores_2d, dtype=np.float32)
    noise = np.ascontiguousarray(noise, dtype=np.float32)
    in_maps = [
        {"x": x_high[i], "sc": scores_2d[i], "nz": noise[i]}
        for i in range(NB)
    ]
    res = bass_utils.run_bass_kernel_spmd(
        nc, in_maps, core_ids=list(range(NB)), trace=trace)
    out = np.concatenate(
        [_unscramble(np.asarray(res.results[i]["o"]))[None] for i in range(NB)],
        axis=0).reshape(NB * K, C, PATCH, PATCH)
    return out, res


def kernel(x_high, scores_2d, noise):
    out, _ = _run(x_high, scores_2d, noise, trace=False)
    return out
